# revision 38
# baseline (speedup 1.0000x reference)
"""
BatchHardContrastiveLoss kernel for Trainium2 (8 NeuronCores, Bass/Tile).

Math (reference):
    posF0 = F0[pp[:,0]], posF1 = F1[pp[:,1]], sub = xyz0[pp[:,0]]   (P=8192)
    pos_loss = mean(relu(max_d (posF0-posF1)^2 - 0.1)^2)
    mask     = ||sub_i - sub_j|| > 0.15          (false_negative / "far")
    min_neg  = min_j where(mask, ||posF0_i - posF1_j||^2, BIG)
    neg_loss = mean(relu(1.4 - min_neg)^2)
    loss     = pos_loss + neg_loss

Device strategy (row-sharded across 8 cores, 1024 rows each):
  * feature distances by GEMM with the norms folded into two extra
    contraction rows:  psum_f[i,j] = d2_ij / 2          (fp16 in, fp32 acc)
  * xyz mask by a second GEMM computing psum_x[i,j] = T^2 - d2xyz_ij
    with hi/lo fp16 splitting of every term (catastrophic-cancellation
    safe); its SIGN is the mask.
  * ScalarE Sign() turns psum_x into pen in {-1, 0, +1}.
  * One fused VectorE tensor_tensor_reduce per [128,1024] tile:
        row_min = min-reduce( max(psum_f, pen) )   chained across tiles.
    Far pair:  max(d2/2, -1) = d2/2  (kept).
    Near pair: max(d2/2, +1) >= 1, and since loss = relu(1.4 - 2*min)^2
    any value >= 0.7 contributes exactly 0 - i.e. masked out, unless the
    row min is a genuine far pair below 0.7 which is then returned exactly.
  * relu^2 + row sums on device; host adds the 8x[128,2] partials.
"""

import sys

import numpy as np

try:
    import ml_dtypes
except ImportError:
    ml_dtypes = None

_TRN_REPO = "/opt/trn_rl_repo"
if _TRN_REPO not in sys.path:
    sys.path.insert(0, _TRN_REPO)

N = 100000
D = 32
P = 8192
NCORES = 8
R = P // NCORES            # rows per core (1024)
POS_THRESH = 0.1
NEG_THRESH = 1.4
MIN_DIST = 0.15
T2 = MIN_DIST * MIN_DIST - 1e-7   # far  <=>  d2xyz > T2

KF = D + 2                 # feature contraction: [-f0, na/2, 1]
KX = 33                    # xyz contraction: 13 hi/lo-split terms zero-padded
                           # to 33 so the xyz GEMM shares the PE's 64-row
                           # tiling mode with the feature GEMM (mixed tiling
                           # modes force TensorE drains between matmuls)


def _build_program(n_rows=R, n_cols=P, cgw=1024):
    """Build the per-core Bass program. All cores run the same program
    (SPMD); per-core data arrives via the input map."""
    from contextlib import ExitStack

    import concourse.bass as bass
    import concourse.mybir as mybir
    import concourse.tile as tile
    from concourse import bacc

    f16 = mybir.dt.float16
    f32 = mybir.dt.float32
    bf16 = mybir.dt.bfloat16
    AF = mybir.ActivationFunctionType
    OP = mybir.AluOpType

    n_rb = n_rows // 128           # row blocks of 128
    n_cg = n_cols // cgw           # column groups
    n_h = cgw // 512               # matmuls (psum banks) per column group
    pos_w = (n_rows // 128) * D    # free dim of the pos-pair slabs

    nc = bacc.Bacc("TRN2", target_bir_lowering=False, debug=False)
    aft = nc.declare_dram_parameter("aft", [KF, n_rows], f16, isOutput=False)
    axt = nc.declare_dram_parameter("axt", [KX, n_rows], f16, isOutput=False)
    bft = nc.declare_dram_parameter("bft", [KF, n_cols], f16, isOutput=False)
    bxt = nc.declare_dram_parameter("bxt", [KX, n_cols], f16, isOutput=False)
    pos = nc.declare_dram_parameter("pos", [128, 2 * pos_w], f32, isOutput=False)
    out = nc.declare_dram_parameter("out", [128, 2], f32, isOutput=True)

    with ExitStack() as ctx:
        tc = ctx.enter_context(tile.TileContext(nc))
        cpool = ctx.enter_context(tc.tile_pool(name="consts", bufs=1))
        spool = ctx.enter_context(tc.tile_pool(name="sign", bufs=4))
        wpool = ctx.enter_context(tc.tile_pool(name="work", bufs=2))
        px_pool = ctx.enter_context(
            tc.tile_pool(name="px", bufs=2, space=bass.MemorySpace.PSUM)
        )
        pf_pool = ctx.enter_context(
            tc.tile_pool(name="pf", bufs=2, space=bass.MemorySpace.PSUM)
        )

        # xyz operands live at SBUF partitions 64.. so their matmuls occupy
        # PE array rows 64+ (tile_position (64,0)) and run CONCURRENTLY with
        # the feature matmuls in rows 0-33 (different row groups).
        # B matrices are split so the first column groups land before the
        # rest and the main loop starts as early as possible.
        SPLIT = min(2048, n_cols)
        aft_s = cpool.tile([KF, n_rows], f16, tag="aft")
        axt_f = cpool.tile([64 + KX, n_rows], f16, tag="axt")
        bft_a = cpool.tile([KF, SPLIT], f16, tag="bft_a")
        bxt_fa = cpool.tile([64 + KX, SPLIT], f16, tag="bxt_a")
        axt_s = axt_f[64:64 + KX, :]
        pos_s = cpool.tile([128, 2 * pos_w], f32, tag="pos")
        nc.sync.dma_start(aft_s[:], aft[:])
        nc.sync.dma_start(axt_s, axt[:])
        nc.scalar.dma_start(bft_a[:], bft[:, 0:SPLIT])
        nc.scalar.dma_start(bxt_fa[64:64 + KX, :], bxt[:, 0:SPLIT])
        if n_cols > SPLIT:
            bft_b = cpool.tile([KF, n_cols - SPLIT], f16, tag="bft_b")
            bxt_fb = cpool.tile([64 + KX, n_cols - SPLIT], f16, tag="bxt_b")
            nc.scalar.dma_start(bft_b[:], bft[:, SPLIT:])
            nc.scalar.dma_start(bxt_fb[64:64 + KX, :], bxt[:, SPLIT:])
        nc.gpsimd.dma_start(pos_s[:], pos[:])

        def bft_sl(c0, w):
            if c0 < SPLIT:
                return bft_a[:, c0:c0 + w]
            return bft_b[:, c0 - SPLIT:c0 - SPLIT + w]

        def bxt_sl(c0, w):
            if c0 < SPLIT:
                return bxt_fa[64:64 + KX, c0:c0 + w]
            return bxt_fb[64:64 + KX, c0 - SPLIT:c0 - SPLIT + w]

        minv = cpool.tile([128, n_rb], f32, tag="minv")

        # bias constants for ScalarE activations
        c_zero = cpool.tile([128, 1], f32, tag="c_zero")
        c_neg_thresh = cpool.tile([128, 1], f32, tag="c_neg_thresh")
        c_pos_thresh = cpool.tile([128, 1], f32, tag="c_pos_thresh")
        nc.vector.memset(c_zero[:], 0.0)
        nc.vector.memset(c_neg_thresh[:], NEG_THRESH)
        nc.vector.memset(c_pos_thresh[:], -POS_THRESH)

        from concourse.masks import make_identity

        eye_s = cpool.tile([128, 128], bf16, tag="eye")
        make_identity(nc, eye_s[:])

        # positive term: relu(max_d (f0-f1)^2 - 0.1)^2, summed
        dif = cpool.tile([128, pos_w], f32, tag="dif")
        nc.vector.tensor_sub(dif[:], pos_s[:, 0:pos_w], pos_s[:, pos_w:2 * pos_w])
        sq = cpool.tile([128, pos_w], f32, tag="sq")
        nc.scalar.activation(sq[:], dif[:], AF.Square, bias=c_zero[:])
        fp = cpool.tile([128, n_rb], f32, tag="fp")
        nc.vector.reduce_max(
            fp[:], sq[:].rearrange("p (a b) -> p a b", b=D),
            axis=mybir.AxisListType.X,
        )
        y1 = cpool.tile([128, n_rb], f32, tag="y1")
        nc.scalar.activation(y1[:], fp[:], AF.Relu, bias=c_pos_thresh[:], scale=1.0)
        y1s = cpool.tile([128, n_rb], f32, tag="y1s")
        nc.scalar.activation(y1s[:], y1[:], AF.Square, bias=c_zero[:])
        possum = cpool.tile([128, 1], f32, tag="possum")
        nc.vector.reduce_sum(possum[:], y1s[:], axis=mybir.AxisListType.X)


        # Hybrid mask application, balancing PE vs DVE:
        #  * plan-1 cgs: pen = Relu(psum_x) accumulated into psum_f by an
        #    identity matmul (PE); DVE only does the min-reduce.
        #  * plan-2 cgs (PLAN2 of every n_cg): no identity matmul; ScalarE
        #    emits Sign(psum_x) in {-1,0,+1} and DVE computes
        #    min-reduce(max(psum_f, sign)) in two passes.
        PLAN2 = (1, 3, 5, 7)
        # cg outer / rb inner: the first column groups (pre-landed SPLIT
        # chunk) supply enough work to hide the remaining B-matrix DMAs.
        acc_all = cpool.tile([128, n_rb * n_cg], f32, tag="acc_all")
        for cg in range(n_cg):
            plan2 = (cg % n_cg) in PLAN2 and n_cg == 8
            for rb in range(n_rb):
                lhs_f = aft_s[:, rb * 128:(rb + 1) * 128]
                lhs_x = axt_f[64:64 + KX, rb * 128:(rb + 1) * 128]
                accum = acc_all[:, rb * n_cg + cg:rb * n_cg + cg + 1]
                px_t = px_pool.tile([128, cgw], f32, tag="px")
                pf_t = pf_pool.tile([128, cgw], f32, tag="pf")
                for h in range(n_h):
                    c0 = cg * cgw + h * 512
                    nc.tensor.matmul(
                        px_t[:, h * 512:(h + 1) * 512],
                        lhs_x, bxt_sl(c0, 512),
                        start=True, stop=True,
                    )
                for h in range(n_h):
                    c0 = cg * cgw + h * 512
                    nc.tensor.matmul(
                        pf_t[:, h * 512:(h + 1) * 512],
                        lhs_f, bft_sl(c0, 512),
                        start=True, stop=plan2,
                    )
                sx = spool.tile([128, cgw], bf16, tag="sx")
                if plan2:
                    # sign in {-1,0,+1}: max(d2/2, +1) >= 1 masks near pairs
                    # (loss relu-saturates below 0.7), max(d2/2, -1) = d2/2
                    nc.scalar.activation(sx[:], px_t[:], AF.Sign, bias=c_zero[:])
                    sel = spool.tile([128, cgw], bf16, tag="sel")
                    nc.vector.tensor_max(sel[:], pf_t[:], sx[:])
                    nc.vector.tensor_reduce(
                        accum, sel[:],
                        axis=mybir.AxisListType.X, op=OP.min,
                    )
                else:
                    # pen = relu(lambda*(T2 - d2xyz)): 0 for far pairs,
                    # lambda*margin for near pairs (>=1.4 well below the
                    # fp16 noise floor of the xyz GEMM)
                    nc.scalar.activation(sx[:], px_t[:], AF.Relu, bias=c_zero[:])
                    for h in range(n_h):
                        nc.tensor.matmul(
                            pf_t[:, h * 512:(h + 1) * 512],
                            eye_s[:], sx[:, h * 512:(h + 1) * 512],
                            start=False, stop=True,
                        )
                    nc.vector.tensor_reduce(
                        accum, pf_t[:],
                        axis=mybir.AxisListType.X, op=OP.min,
                    )
        nc.vector.tensor_reduce(
            minv[:], acc_all[:].rearrange("p (a b) -> p a b", b=n_cg),
            axis=mybir.AxisListType.X, op=OP.min,
        )

        # negative term: relu(1.4 - 2*minv)^2, summed over this core's rows
        y2 = cpool.tile([128, n_rb], f32, tag="y2")
        nc.scalar.activation(y2[:], minv[:], AF.Relu, bias=c_neg_thresh[:], scale=-2.0)
        y2s = cpool.tile([128, n_rb], f32, tag="y2s")
        nc.scalar.activation(y2s[:], y2[:], AF.Square, bias=c_zero[:])
        negsum = cpool.tile([128, 1], f32, tag="negsum")
        nc.vector.reduce_sum(negsum[:], y2s[:], axis=mybir.AxisListType.X)

        outt = cpool.tile([128, 2], f32, tag="outt")
        nc.vector.tensor_copy(outt[:, 0:1], possum[:])
        nc.vector.tensor_copy(outt[:, 1:2], negsum[:])
        nc.gpsimd.dma_start(out[:], outt[:])

    nc.compile()
    return nc


def _split16(v):
    """hi/lo split: v ~= hi + lo with both exactly representable in fp16."""
    hi = v.astype(np.float16)
    lo = (v.astype(np.float32) - hi.astype(np.float32)).astype(np.float16)
    return hi, lo


def _host_prep(F0, F1, positive_pairs, xyz0):
    """Gather + build the augmented GEMM operands (float16)."""
    idx0 = np.asarray(positive_pairs)[:, 0].astype(np.int64)
    idx1 = np.asarray(positive_pairs)[:, 1].astype(np.int64)
    posF0 = np.asarray(F0, dtype=np.float32)[idx0]        # [P, D]
    posF1 = np.asarray(F1, dtype=np.float32)[idx1]        # [P, D]
    sub = np.asarray(xyz0, dtype=np.float32)[idx0]        # [P, 3]
    npairs = posF0.shape[0]

    na = (posF0.astype(np.float64) ** 2).sum(1).astype(np.float32)
    nb = (posF1.astype(np.float64) ** 2).sum(1).astype(np.float32)
    xn = (sub.astype(np.float64) ** 2).sum(1).astype(np.float32)

    # feature GEMM: psum_f[i,j] = d2_ij / 2 = na/2 + nb/2 - f0.f1
    Af = np.concatenate(
        [-posF0, (na / 2)[:, None], np.ones((npairs, 1), np.float32)], axis=1
    )  # [P, 34]
    Bf = np.concatenate(
        [posF1, np.ones((npairs, 1), np.float32), (nb / 2)[:, None]], axis=1
    )  # [P, 34]

    # xyz GEMM: psum_x[i,j] = T2 - d2xyz_ij
    #         = (T2 - xn_i) - xn_j + 2*x_i.x_j, each term hi/lo split in fp16:
    # K layout (13): [2*x_hi(3), 2*x_hi(3), 2*x_lo(3), rowc_hi, rowc_lo, 1, 1]
    #     against   [  y_hi(3),   y_lo(3),   y_hi(3),    1,       1, -xn_hi, -xn_lo]
    x_hi, x_lo = _split16(sub)
    rowc = T2 - xn
    rowc_hi, rowc_lo = _split16(rowc)
    xn_hi, xn_lo = _split16(xn)
    ones = np.ones((npairs, 1), np.float32)
    Ax = np.concatenate(
        [
            2.0 * x_hi.astype(np.float32),
            2.0 * x_hi.astype(np.float32),
            2.0 * x_lo.astype(np.float32),
            rowc_hi.astype(np.float32)[:, None],
            rowc_lo.astype(np.float32)[:, None],
            ones,
            ones,
        ],
        axis=1,
    )  # [P, 13]
    Bx = np.concatenate(
        [
            x_hi.astype(np.float32),
            x_lo.astype(np.float32),
            x_hi.astype(np.float32),
            ones,
            ones,
            -xn_hi.astype(np.float32)[:, None],
            -xn_lo.astype(np.float32)[:, None],
        ],
        axis=1,
    )  # [P, 13]

    pad = np.zeros((npairs, KX - Ax.shape[1]), np.float32)
    Ax = np.concatenate([Ax, pad], axis=1)                # [P, KX]
    Bx = np.concatenate([Bx, pad], axis=1)

    AfT = np.ascontiguousarray(Af.T).astype(np.float16)   # [34, P]
    BfT = np.ascontiguousarray(Bf.T).astype(np.float16)
    AxT = np.ascontiguousarray(Ax.T).astype(np.float16)   # [KX, P]
    BxT = np.ascontiguousarray(Bx.T).astype(np.float16)
    return AfT, BfT, AxT, BxT, posF0, posF1


def _pos_slab(arr, c):
    """[R, D] slab for core c -> [128, (R/128)*D] with row r = rb*128 + p
    mapped to partition p, columns rb*D..rb*D+D."""
    slab = arr[c * R:(c + 1) * R]                          # [R, D]
    return np.ascontiguousarray(
        slab.reshape(R // 128, 128, D).transpose(1, 0, 2).reshape(128, -1)
    ).astype(np.float32)


_LDW_OPT_PATCHED = False


def _enable_ldw_opt():
    """Ask walrus to dedupe/hoist redundant LDWEIGHTS (off by default in
    this harness); correctness is re-checked against the reference on every
    run."""
    global _LDW_OPT_PATCHED
    if _LDW_OPT_PATCHED:
        return
    from concourse import bass_utils as _bu

    _orig = _bu.run_command

    def _patched(cmd, *a, **k):
        if isinstance(cmd, list):
            cmd = [
                "--enable-ldw-opt=true" if c == "--enable-ldw-opt=false" else c
                for c in cmd
            ]
        return _orig(cmd, *a, **k)

    _bu.run_command = _patched
    _LDW_OPT_PATCHED = True


def kernel(F0, F1, positive_pairs, xyz0):
    from concourse.bass_utils import run_bass_kernel_spmd

    AfT, BfT, AxT, BxT, posF0, posF1 = _host_prep(F0, F1, positive_pairs, xyz0)

    nc = _build_program()

    in_maps = []
    for c in range(NCORES):
        in_maps.append(
            {
                "aft": np.ascontiguousarray(AfT[:, c * R:(c + 1) * R]),
                "axt": np.ascontiguousarray(AxT[:, c * R:(c + 1) * R]),
                "bft": BfT,
                "bxt": BxT,
                "pos": np.ascontiguousarray(
                    np.concatenate([_pos_slab(posF0, c), _pos_slab(posF1, c)], axis=1)
                ),
            }
        )

    res = run_bass_kernel_spmd(nc, in_maps, list(range(NCORES)))
    globals()["_LAST_RESULTS"] = res
    total = 0.0
    for r in res.results:
        o = r["out"].astype(np.float64)
        total += o[:, 0].sum() + o[:, 1].sum()
    return np.float32(total / P)


# revision 39
# speedup vs baseline: 1.0122x; 1.0122x over previous
"""
BatchHardContrastiveLoss kernel for Trainium2 (8 NeuronCores, Bass/Tile).

Math (reference):
    posF0 = F0[pp[:,0]], posF1 = F1[pp[:,1]], sub = xyz0[pp[:,0]]   (P=8192)
    pos_loss = mean(relu(max_d (posF0-posF1)^2 - 0.1)^2)
    mask     = ||sub_i - sub_j|| > 0.15          (false_negative / "far")
    min_neg  = min_j where(mask, ||posF0_i - posF1_j||^2, BIG)
    neg_loss = mean(relu(1.4 - min_neg)^2)
    loss     = pos_loss + neg_loss

Device strategy (row-sharded across 8 cores, 1024 rows each):
  * feature distances by GEMM with the norms folded into two extra
    contraction rows:  psum_f[i,j] = d2_ij / 2          (fp16 in, fp32 acc)
  * xyz mask by a second GEMM computing psum_x[i,j] = T^2 - d2xyz_ij
    with hi/lo fp16 splitting of every term (catastrophic-cancellation
    safe); its SIGN is the mask.
  * ScalarE Sign() turns psum_x into pen in {-1, 0, +1}.
  * One fused VectorE tensor_tensor_reduce per [128,1024] tile:
        row_min = min-reduce( max(psum_f, pen) )   chained across tiles.
    Far pair:  max(d2/2, -1) = d2/2  (kept).
    Near pair: max(d2/2, +1) >= 1, and since loss = relu(1.4 - 2*min)^2
    any value >= 0.7 contributes exactly 0 - i.e. masked out, unless the
    row min is a genuine far pair below 0.7 which is then returned exactly.
  * relu^2 + row sums on device; host adds the 8x[128,2] partials.
"""

import sys

import numpy as np

try:
    import ml_dtypes
except ImportError:
    ml_dtypes = None

_TRN_REPO = "/opt/trn_rl_repo"
if _TRN_REPO not in sys.path:
    sys.path.insert(0, _TRN_REPO)

N = 100000
D = 32
P = 8192
NCORES = 8
R = P // NCORES            # rows per core (1024)
POS_THRESH = 0.1
NEG_THRESH = 1.4
MIN_DIST = 0.15
T2 = MIN_DIST * MIN_DIST - 1e-7   # far  <=>  d2xyz > T2

KF = D + 2                 # feature contraction: [-f0, na/2, 1]
KX = 33                    # xyz contraction: 13 hi/lo-split terms zero-padded
                           # to 33 so the xyz GEMM shares the PE's 64-row
                           # tiling mode with the feature GEMM (mixed tiling
                           # modes force TensorE drains between matmuls)


def _build_program(n_rows=R, n_cols=P, cgw=1024):
    """Build the per-core Bass program. All cores run the same program
    (SPMD); per-core data arrives via the input map."""
    from contextlib import ExitStack

    import concourse.bass as bass
    import concourse.mybir as mybir
    import concourse.tile as tile
    from concourse import bacc

    f16 = mybir.dt.float16
    f32 = mybir.dt.float32
    bf16 = mybir.dt.bfloat16
    AF = mybir.ActivationFunctionType
    OP = mybir.AluOpType

    n_rb = n_rows // 128           # row blocks of 128
    n_cg = n_cols // cgw           # column groups
    n_h = cgw // 512               # matmuls (psum banks) per column group
    pos_w = (n_rows // 128) * D    # free dim of the pos-pair slabs

    nc = bacc.Bacc("TRN2", target_bir_lowering=False, debug=False)
    aft = nc.declare_dram_parameter("aft", [KF, n_rows], f16, isOutput=False)
    axt = nc.declare_dram_parameter("axt", [KX, n_rows], f16, isOutput=False)
    bft = nc.declare_dram_parameter("bft", [KF, n_cols], f16, isOutput=False)
    bxt = nc.declare_dram_parameter("bxt", [KX, n_cols], f16, isOutput=False)
    pos = nc.declare_dram_parameter("pos", [128, 2 * pos_w], f32, isOutput=False)
    out = nc.declare_dram_parameter("out", [128, 2], f32, isOutput=True)

    with ExitStack() as ctx:
        tc = ctx.enter_context(tile.TileContext(nc))
        cpool = ctx.enter_context(tc.tile_pool(name="consts", bufs=1))
        spool = ctx.enter_context(tc.tile_pool(name="sign", bufs=4))
        wpool = ctx.enter_context(tc.tile_pool(name="work", bufs=2))
        px_pool = ctx.enter_context(
            tc.tile_pool(name="px", bufs=2, space=bass.MemorySpace.PSUM)
        )
        pf_pool = ctx.enter_context(
            tc.tile_pool(name="pf", bufs=2, space=bass.MemorySpace.PSUM)
        )

        # xyz operands live at SBUF partitions 64.. so their matmuls occupy
        # PE array rows 64+ (tile_position (64,0)) and run CONCURRENTLY with
        # the feature matmuls in rows 0-33 (different row groups).
        # B matrices are split so the first column groups land before the
        # rest and the main loop starts as early as possible.
        SPLIT = min(2048, n_cols)
        aft_s = cpool.tile([KF, n_rows], f16, tag="aft")
        axt_f = cpool.tile([64 + KX, n_rows], f16, tag="axt")
        bft_a = cpool.tile([KF, SPLIT], f16, tag="bft_a")
        bxt_fa = cpool.tile([64 + KX, SPLIT], f16, tag="bxt_a")
        axt_s = axt_f[64:64 + KX, :]
        pos_s = cpool.tile([128, 2 * pos_w], f32, tag="pos")
        nc.sync.dma_start(aft_s[:], aft[:])
        nc.sync.dma_start(axt_s, axt[:])
        nc.scalar.dma_start(bft_a[:], bft[:, 0:SPLIT])
        nc.scalar.dma_start(bxt_fa[64:64 + KX, :], bxt[:, 0:SPLIT])
        if n_cols > SPLIT:
            bft_b = cpool.tile([KF, n_cols - SPLIT], f16, tag="bft_b")
            bxt_fb = cpool.tile([64 + KX, n_cols - SPLIT], f16, tag="bxt_b")
            nc.scalar.dma_start(bft_b[:], bft[:, SPLIT:])
            nc.scalar.dma_start(bxt_fb[64:64 + KX, :], bxt[:, SPLIT:])
        nc.gpsimd.dma_start(pos_s[:], pos[:])

        def bft_sl(c0, w):
            if c0 < SPLIT:
                return bft_a[:, c0:c0 + w]
            return bft_b[:, c0 - SPLIT:c0 - SPLIT + w]

        def bxt_sl(c0, w):
            if c0 < SPLIT:
                return bxt_fa[64:64 + KX, c0:c0 + w]
            return bxt_fb[64:64 + KX, c0 - SPLIT:c0 - SPLIT + w]

        minv = cpool.tile([128, n_rb], f32, tag="minv")

        # bias constants for ScalarE activations
        c_zero = cpool.tile([128, 1], f32, tag="c_zero")
        c_neg_thresh = cpool.tile([128, 1], f32, tag="c_neg_thresh")
        c_pos_thresh = cpool.tile([128, 1], f32, tag="c_pos_thresh")
        nc.vector.memset(c_zero[:], 0.0)
        nc.vector.memset(c_neg_thresh[:], NEG_THRESH)
        nc.vector.memset(c_pos_thresh[:], -POS_THRESH)

        from concourse.masks import make_identity

        eye_s = cpool.tile([128, 128], bf16, tag="eye")
        make_identity(nc, eye_s[:])

        # positive term: relu(max_d (f0-f1)^2 - 0.1)^2, summed
        dif = cpool.tile([128, pos_w], f32, tag="dif")
        nc.vector.tensor_sub(dif[:], pos_s[:, 0:pos_w], pos_s[:, pos_w:2 * pos_w])
        sq = cpool.tile([128, pos_w], f32, tag="sq")
        nc.scalar.activation(sq[:], dif[:], AF.Square, bias=c_zero[:])
        fp = cpool.tile([128, n_rb], f32, tag="fp")
        nc.vector.reduce_max(
            fp[:], sq[:].rearrange("p (a b) -> p a b", b=D),
            axis=mybir.AxisListType.X,
        )
        y1 = cpool.tile([128, n_rb], f32, tag="y1")
        nc.scalar.activation(y1[:], fp[:], AF.Relu, bias=c_pos_thresh[:], scale=1.0)
        y1s = cpool.tile([128, n_rb], f32, tag="y1s")
        nc.scalar.activation(y1s[:], y1[:], AF.Square, bias=c_zero[:])
        possum = cpool.tile([128, 1], f32, tag="possum")
        nc.vector.reduce_sum(possum[:], y1s[:], axis=mybir.AxisListType.X)


        # Hybrid mask application, balancing PE vs DVE:
        #  * plan-1 cgs: pen = Relu(psum_x) accumulated into psum_f by an
        #    identity matmul (PE); DVE only does the min-reduce.
        #  * plan-2 cgs (PLAN2 of every n_cg): no identity matmul; ScalarE
        #    emits Sign(psum_x) in {-1,0,+1} and DVE computes
        #    min-reduce(max(psum_f, sign)) in two passes.
        PLAN2 = (1, 3, 5)
        # cg outer / rb inner: the first column groups (pre-landed SPLIT
        # chunk) supply enough work to hide the remaining B-matrix DMAs.
        acc_all = cpool.tile([128, n_rb * n_cg], f32, tag="acc_all")
        for cg in range(n_cg):
            plan2 = (cg % n_cg) in PLAN2 and n_cg == 8
            for rb in range(n_rb):
                lhs_f = aft_s[:, rb * 128:(rb + 1) * 128]
                lhs_x = axt_f[64:64 + KX, rb * 128:(rb + 1) * 128]
                accum = acc_all[:, rb * n_cg + cg:rb * n_cg + cg + 1]
                px_t = px_pool.tile([128, cgw], f32, tag="px")
                pf_t = pf_pool.tile([128, cgw], f32, tag="pf")
                for h in range(n_h):
                    c0 = cg * cgw + h * 512
                    nc.tensor.matmul(
                        px_t[:, h * 512:(h + 1) * 512],
                        lhs_x, bxt_sl(c0, 512),
                        start=True, stop=True,
                    )
                for h in range(n_h):
                    c0 = cg * cgw + h * 512
                    nc.tensor.matmul(
                        pf_t[:, h * 512:(h + 1) * 512],
                        lhs_f, bft_sl(c0, 512),
                        start=True, stop=plan2,
                    )
                sx = spool.tile([128, cgw], bf16, tag="sx")
                if plan2:
                    # sign in {-1,0,+1}: max(d2/2, +1) >= 1 masks near pairs
                    # (loss relu-saturates below 0.7), max(d2/2, -1) = d2/2
                    nc.scalar.activation(sx[:], px_t[:], AF.Sign, bias=c_zero[:])
                    sel = spool.tile([128, cgw], bf16, tag="sel")
                    nc.vector.tensor_max(sel[:], pf_t[:], sx[:])
                    nc.vector.tensor_reduce(
                        accum, sel[:],
                        axis=mybir.AxisListType.X, op=OP.min,
                    )
                else:
                    # pen = relu(lambda*(T2 - d2xyz)): 0 for far pairs,
                    # lambda*margin for near pairs (>=1.4 well below the
                    # fp16 noise floor of the xyz GEMM)
                    nc.scalar.activation(sx[:], px_t[:], AF.Relu, bias=c_zero[:])
                    for h in range(n_h):
                        nc.tensor.matmul(
                            pf_t[:, h * 512:(h + 1) * 512],
                            eye_s[:], sx[:, h * 512:(h + 1) * 512],
                            start=False, stop=True,
                        )
                    nc.vector.tensor_reduce(
                        accum, pf_t[:],
                        axis=mybir.AxisListType.X, op=OP.min,
                    )
        nc.vector.tensor_reduce(
            minv[:], acc_all[:].rearrange("p (a b) -> p a b", b=n_cg),
            axis=mybir.AxisListType.X, op=OP.min,
        )

        # negative term: relu(1.4 - 2*minv)^2, summed over this core's rows
        y2 = cpool.tile([128, n_rb], f32, tag="y2")
        nc.scalar.activation(y2[:], minv[:], AF.Relu, bias=c_neg_thresh[:], scale=-2.0)
        y2s = cpool.tile([128, n_rb], f32, tag="y2s")
        nc.scalar.activation(y2s[:], y2[:], AF.Square, bias=c_zero[:])
        negsum = cpool.tile([128, 1], f32, tag="negsum")
        nc.vector.reduce_sum(negsum[:], y2s[:], axis=mybir.AxisListType.X)

        outt = cpool.tile([128, 2], f32, tag="outt")
        nc.vector.tensor_copy(outt[:, 0:1], possum[:])
        nc.vector.tensor_copy(outt[:, 1:2], negsum[:])
        nc.gpsimd.dma_start(out[:], outt[:])

    nc.compile()
    return nc


def _split16(v):
    """hi/lo split: v ~= hi + lo with both exactly representable in fp16."""
    hi = v.astype(np.float16)
    lo = (v.astype(np.float32) - hi.astype(np.float32)).astype(np.float16)
    return hi, lo


def _host_prep(F0, F1, positive_pairs, xyz0):
    """Gather + build the augmented GEMM operands (float16)."""
    idx0 = np.asarray(positive_pairs)[:, 0].astype(np.int64)
    idx1 = np.asarray(positive_pairs)[:, 1].astype(np.int64)
    posF0 = np.asarray(F0, dtype=np.float32)[idx0]        # [P, D]
    posF1 = np.asarray(F1, dtype=np.float32)[idx1]        # [P, D]
    sub = np.asarray(xyz0, dtype=np.float32)[idx0]        # [P, 3]
    npairs = posF0.shape[0]

    na = (posF0.astype(np.float64) ** 2).sum(1).astype(np.float32)
    nb = (posF1.astype(np.float64) ** 2).sum(1).astype(np.float32)
    xn = (sub.astype(np.float64) ** 2).sum(1).astype(np.float32)

    # feature GEMM: psum_f[i,j] = d2_ij / 2 = na/2 + nb/2 - f0.f1
    Af = np.concatenate(
        [-posF0, (na / 2)[:, None], np.ones((npairs, 1), np.float32)], axis=1
    )  # [P, 34]
    Bf = np.concatenate(
        [posF1, np.ones((npairs, 1), np.float32), (nb / 2)[:, None]], axis=1
    )  # [P, 34]

    # xyz GEMM: psum_x[i,j] = T2 - d2xyz_ij
    #         = (T2 - xn_i) - xn_j + 2*x_i.x_j, each term hi/lo split in fp16:
    # K layout (13): [2*x_hi(3), 2*x_hi(3), 2*x_lo(3), rowc_hi, rowc_lo, 1, 1]
    #     against   [  y_hi(3),   y_lo(3),   y_hi(3),    1,       1, -xn_hi, -xn_lo]
    x_hi, x_lo = _split16(sub)
    rowc = T2 - xn
    rowc_hi, rowc_lo = _split16(rowc)
    xn_hi, xn_lo = _split16(xn)
    ones = np.ones((npairs, 1), np.float32)
    Ax = np.concatenate(
        [
            2.0 * x_hi.astype(np.float32),
            2.0 * x_hi.astype(np.float32),
            2.0 * x_lo.astype(np.float32),
            rowc_hi.astype(np.float32)[:, None],
            rowc_lo.astype(np.float32)[:, None],
            ones,
            ones,
        ],
        axis=1,
    )  # [P, 13]
    Bx = np.concatenate(
        [
            x_hi.astype(np.float32),
            x_lo.astype(np.float32),
            x_hi.astype(np.float32),
            ones,
            ones,
            -xn_hi.astype(np.float32)[:, None],
            -xn_lo.astype(np.float32)[:, None],
        ],
        axis=1,
    )  # [P, 13]

    pad = np.zeros((npairs, KX - Ax.shape[1]), np.float32)
    Ax = np.concatenate([Ax, pad], axis=1)                # [P, KX]
    Bx = np.concatenate([Bx, pad], axis=1)

    AfT = np.ascontiguousarray(Af.T).astype(np.float16)   # [34, P]
    BfT = np.ascontiguousarray(Bf.T).astype(np.float16)
    AxT = np.ascontiguousarray(Ax.T).astype(np.float16)   # [KX, P]
    BxT = np.ascontiguousarray(Bx.T).astype(np.float16)
    return AfT, BfT, AxT, BxT, posF0, posF1


def _pos_slab(arr, c):
    """[R, D] slab for core c -> [128, (R/128)*D] with row r = rb*128 + p
    mapped to partition p, columns rb*D..rb*D+D."""
    slab = arr[c * R:(c + 1) * R]                          # [R, D]
    return np.ascontiguousarray(
        slab.reshape(R // 128, 128, D).transpose(1, 0, 2).reshape(128, -1)
    ).astype(np.float32)


_LDW_OPT_PATCHED = False


def _enable_ldw_opt():
    """Ask walrus to dedupe/hoist redundant LDWEIGHTS (off by default in
    this harness); correctness is re-checked against the reference on every
    run."""
    global _LDW_OPT_PATCHED
    if _LDW_OPT_PATCHED:
        return
    from concourse import bass_utils as _bu

    _orig = _bu.run_command

    def _patched(cmd, *a, **k):
        if isinstance(cmd, list):
            cmd = [
                "--enable-ldw-opt=true" if c == "--enable-ldw-opt=false" else c
                for c in cmd
            ]
        return _orig(cmd, *a, **k)

    _bu.run_command = _patched
    _LDW_OPT_PATCHED = True


def kernel(F0, F1, positive_pairs, xyz0):
    from concourse.bass_utils import run_bass_kernel_spmd

    AfT, BfT, AxT, BxT, posF0, posF1 = _host_prep(F0, F1, positive_pairs, xyz0)

    nc = _build_program()

    in_maps = []
    for c in range(NCORES):
        in_maps.append(
            {
                "aft": np.ascontiguousarray(AfT[:, c * R:(c + 1) * R]),
                "axt": np.ascontiguousarray(AxT[:, c * R:(c + 1) * R]),
                "bft": BfT,
                "bxt": BxT,
                "pos": np.ascontiguousarray(
                    np.concatenate([_pos_slab(posF0, c), _pos_slab(posF1, c)], axis=1)
                ),
            }
        )

    res = run_bass_kernel_spmd(nc, in_maps, list(range(NCORES)))
    globals()["_LAST_RESULTS"] = res
    total = 0.0
    for r in res.results:
        o = r["out"].astype(np.float64)
        total += o[:, 0].sum() + o[:, 1].sum()
    return np.float32(total / P)


# revision 40
# speedup vs baseline: 1.0326x; 1.0202x over previous
"""
BatchHardContrastiveLoss kernel for Trainium2 (8 NeuronCores, Bass/Tile).

Math (reference):
    posF0 = F0[pp[:,0]], posF1 = F1[pp[:,1]], sub = xyz0[pp[:,0]]   (P=8192)
    pos_loss = mean(relu(max_d (posF0-posF1)^2 - 0.1)^2)
    far      = ||sub_i - sub_j|| > 0.15          (false-negative mask)
    min_neg  = min_j where(far, ||posF0_i - posF1_j||^2, BIG)
    neg_loss = mean(relu(1.4 - min_neg)^2)
    loss     = pos_loss + neg_loss

Device strategy (rows sharded across 8 cores, 1024 rows/core; each core
computes its [1024, 8192] distance slab against replicated operands):
  * feature GEMM (fp16 in / fp32 acc, K=34) with both norms folded in as
    extra contraction rows: psum_f[i,j] = d2_ij / 2.
  * xyz GEMM (K=33) computes psum_x[i,j] = lambda*(T^2 - d2xyz_ij) with
    every term hi/lo-split in fp16 (catastrophic-cancellation safe).
    Both GEMMs use the PE's 64-row tiling mode at different row groups
    (feat rows 0-33, xyz rows 64-96) so they execute CONCURRENTLY.
  * mask + row-min, balanced across engines per column group:
      plan-1: ScalarE pen = Relu(psum_x) (0 for far pairs, >= 1.4 for
        near pairs away from the fp16 noise floor); TensorE accumulates
        pen into psum_f via an identity matmul; VectorE does one
        min-reduce.  (PE-heavy, DVE-light)
      plan-2: ScalarE Sign(psum_x) in {-1,0,+1}; VectorE computes
        min-reduce(max(psum_f, sign)).  max(d2/2,-1)=d2/2 keeps far
        pairs; near pairs become >= 1 and the loss relu saturates to 0
        for any row-min >= 0.7, so they can never affect the loss.
        (PE-light, DVE-heavy)
  * relu^2 + row sums on device; host sums the 8x[128,2] partials.
"""

import sys

import numpy as np

try:
    import ml_dtypes
except ImportError:
    ml_dtypes = None

_TRN_REPO = "/opt/trn_rl_repo"
if _TRN_REPO not in sys.path:
    sys.path.insert(0, _TRN_REPO)

N = 100000
D = 32
P = 8192
NCORES = 8
R = P // NCORES            # rows per core (1024)
POS_THRESH = 0.1
NEG_THRESH = 1.4
MIN_DIST = 0.15
T2 = MIN_DIST * MIN_DIST - 1e-7   # far  <=>  d2xyz > T2

KF = D + 2                 # feature contraction: [-f0, na/2, 1]
KX = 33                    # xyz contraction: 13 hi/lo-split terms zero-padded
                           # to 33 so the xyz GEMM shares the PE's 64-row
                           # tiling mode with the feature GEMM (mixed tiling
                           # modes force TensorE drains between matmuls)


def _build_program(n_rows=R, n_cols=P, cgw=1024):
    """Build the per-core Bass program. All cores run the same program
    (SPMD); per-core data arrives via the input map."""
    from contextlib import ExitStack

    import concourse.bass as bass
    import concourse.mybir as mybir
    import concourse.tile as tile
    from concourse import bacc

    f16 = mybir.dt.float16
    f32 = mybir.dt.float32
    bf16 = mybir.dt.bfloat16
    AF = mybir.ActivationFunctionType
    OP = mybir.AluOpType

    n_rb = n_rows // 128           # row blocks of 128
    n_cg = n_cols // cgw           # column groups
    n_h = cgw // 512               # matmuls (psum banks) per column group
    pos_w = (n_rows // 128) * D    # free dim of the pos-pair slabs

    nc = bacc.Bacc("TRN2", target_bir_lowering=False, debug=False)
    aft = nc.declare_dram_parameter("aft", [KF, n_rows], f16, isOutput=False)
    axt = nc.declare_dram_parameter("axt", [KX, n_rows], f16, isOutput=False)
    bft = nc.declare_dram_parameter("bft", [KF, n_cols], f16, isOutput=False)
    bxt = nc.declare_dram_parameter("bxt", [KX, n_cols], f16, isOutput=False)
    pos = nc.declare_dram_parameter("pos", [128, 2 * pos_w], f32, isOutput=False)
    out = nc.declare_dram_parameter("out", [128, 2], f32, isOutput=True)

    with ExitStack() as ctx:
        tc = ctx.enter_context(tile.TileContext(nc))
        cpool = ctx.enter_context(tc.tile_pool(name="consts", bufs=1))
        spool = ctx.enter_context(tc.tile_pool(name="sign", bufs=4))
        wpool = ctx.enter_context(tc.tile_pool(name="work", bufs=2))
        px_pool = ctx.enter_context(
            tc.tile_pool(name="px", bufs=2, space=bass.MemorySpace.PSUM)
        )
        pf_pool = ctx.enter_context(
            tc.tile_pool(name="pf", bufs=2, space=bass.MemorySpace.PSUM)
        )

        # xyz operands live at SBUF partitions 64.. so their matmuls occupy
        # PE array rows 64+ (tile_position (64,0)) and run CONCURRENTLY with
        # the feature matmuls in rows 0-33 (different row groups).
        # B matrices are split so the first column groups land before the
        # rest and the main loop starts as early as possible.
        SPLIT = min(1024, n_cols)
        aft_s = cpool.tile([KF, n_rows], f16, tag="aft")
        axt_f = cpool.tile([64 + KX, n_rows], f16, tag="axt")
        bft_a = cpool.tile([KF, SPLIT], f16, tag="bft_a")
        bxt_fa = cpool.tile([64 + KX, SPLIT], f16, tag="bxt_a")
        axt_s = axt_f[64:64 + KX, :]
        pos_s = cpool.tile([128, 2 * pos_w], f32, tag="pos")
        nc.sync.dma_start(aft_s[:], aft[:])
        nc.sync.dma_start(axt_s, axt[:])
        nc.scalar.dma_start(bft_a[:], bft[:, 0:SPLIT])
        nc.scalar.dma_start(bxt_fa[64:64 + KX, :], bxt[:, 0:SPLIT])
        if n_cols > SPLIT:
            bft_b = cpool.tile([KF, n_cols - SPLIT], f16, tag="bft_b")
            bxt_fb = cpool.tile([64 + KX, n_cols - SPLIT], f16, tag="bxt_b")
            nc.scalar.dma_start(bft_b[:], bft[:, SPLIT:])
            nc.scalar.dma_start(bxt_fb[64:64 + KX, :], bxt[:, SPLIT:])
        nc.gpsimd.dma_start(pos_s[:], pos[:])

        def bft_sl(c0, w):
            if c0 < SPLIT:
                return bft_a[:, c0:c0 + w]
            return bft_b[:, c0 - SPLIT:c0 - SPLIT + w]

        def bxt_sl(c0, w):
            if c0 < SPLIT:
                return bxt_fa[64:64 + KX, c0:c0 + w]
            return bxt_fb[64:64 + KX, c0 - SPLIT:c0 - SPLIT + w]

        minv = cpool.tile([128, n_rb], f32, tag="minv")

        # bias constants for ScalarE activations
        c_zero = cpool.tile([128, 1], f32, tag="c_zero")
        c_neg_thresh = cpool.tile([128, 1], f32, tag="c_neg_thresh")
        c_pos_thresh = cpool.tile([128, 1], f32, tag="c_pos_thresh")
        nc.vector.memset(c_zero[:], 0.0)
        nc.vector.memset(c_neg_thresh[:], NEG_THRESH)
        nc.vector.memset(c_pos_thresh[:], -POS_THRESH)

        from concourse.masks import make_identity

        eye_s = cpool.tile([128, 128], bf16, tag="eye")
        make_identity(nc, eye_s[:])

        # positive term: relu(max_d (f0-f1)^2 - 0.1)^2, summed
        dif = cpool.tile([128, pos_w], f32, tag="dif")
        nc.vector.tensor_sub(dif[:], pos_s[:, 0:pos_w], pos_s[:, pos_w:2 * pos_w])
        sq = cpool.tile([128, pos_w], f32, tag="sq")
        nc.scalar.activation(sq[:], dif[:], AF.Square, bias=c_zero[:])
        fp = cpool.tile([128, n_rb], f32, tag="fp")
        nc.vector.reduce_max(
            fp[:], sq[:].rearrange("p (a b) -> p a b", b=D),
            axis=mybir.AxisListType.X,
        )
        y1 = cpool.tile([128, n_rb], f32, tag="y1")
        nc.scalar.activation(y1[:], fp[:], AF.Relu, bias=c_pos_thresh[:], scale=1.0)
        y1s = cpool.tile([128, n_rb], f32, tag="y1s")
        nc.scalar.activation(y1s[:], y1[:], AF.Square, bias=c_zero[:])
        possum = cpool.tile([128, 1], f32, tag="possum")
        nc.vector.reduce_sum(possum[:], y1s[:], axis=mybir.AxisListType.X)


        # Hybrid mask application, balancing PE vs DVE:
        #  * plan-1 cgs: pen = Relu(psum_x) accumulated into psum_f by an
        #    identity matmul (PE); DVE only does the min-reduce.
        #  * plan-2 cgs (PLAN2 of every n_cg): no identity matmul; ScalarE
        #    emits Sign(psum_x) in {-1,0,+1} and DVE computes
        #    min-reduce(max(psum_f, sign)) in two passes.
        PLAN2 = (1, 3, 5)
        # cg outer / rb inner: the first column groups (pre-landed SPLIT
        # chunk) supply enough work to hide the remaining B-matrix DMAs.
        acc_all = cpool.tile([128, n_rb * n_cg], f32, tag="acc_all")
        for cg in range(n_cg):
            plan2 = (cg % n_cg) in PLAN2 and n_cg == 8
            for rb in range(n_rb):
                lhs_f = aft_s[:, rb * 128:(rb + 1) * 128]
                lhs_x = axt_f[64:64 + KX, rb * 128:(rb + 1) * 128]
                accum = acc_all[:, rb * n_cg + cg:rb * n_cg + cg + 1]
                px_t = px_pool.tile([128, cgw], f32, tag="px")
                pf_t = pf_pool.tile([128, cgw], f32, tag="pf")
                for h in range(n_h):
                    c0 = cg * cgw + h * 512
                    nc.tensor.matmul(
                        px_t[:, h * 512:(h + 1) * 512],
                        lhs_x, bxt_sl(c0, 512),
                        start=True, stop=True,
                    )
                for h in range(n_h):
                    c0 = cg * cgw + h * 512
                    nc.tensor.matmul(
                        pf_t[:, h * 512:(h + 1) * 512],
                        lhs_f, bft_sl(c0, 512),
                        start=True, stop=plan2,
                    )
                sx = spool.tile([128, cgw], bf16, tag="sx")
                if plan2:
                    # sign in {-1,0,+1}: max(d2/2, +1) >= 1 masks near pairs
                    # (loss relu-saturates below 0.7), max(d2/2, -1) = d2/2
                    nc.scalar.activation(sx[:], px_t[:], AF.Sign, bias=c_zero[:])
                    sel = spool.tile([128, cgw], bf16, tag="sel")
                    nc.vector.tensor_max(sel[:], pf_t[:], sx[:])
                    nc.vector.tensor_reduce(
                        accum, sel[:],
                        axis=mybir.AxisListType.X, op=OP.min,
                    )
                else:
                    # pen = relu(lambda*(T2 - d2xyz)): 0 for far pairs,
                    # lambda*margin for near pairs (>=1.4 well below the
                    # fp16 noise floor of the xyz GEMM)
                    nc.scalar.activation(sx[:], px_t[:], AF.Relu, bias=c_zero[:])
                    for h in range(n_h):
                        nc.tensor.matmul(
                            pf_t[:, h * 512:(h + 1) * 512],
                            eye_s[:], sx[:, h * 512:(h + 1) * 512],
                            start=False, stop=True,
                        )
                    nc.vector.tensor_reduce(
                        accum, pf_t[:],
                        axis=mybir.AxisListType.X, op=OP.min,
                    )
        nc.vector.tensor_reduce(
            minv[:], acc_all[:].rearrange("p (a b) -> p a b", b=n_cg),
            axis=mybir.AxisListType.X, op=OP.min,
        )

        # negative term: relu(1.4 - 2*minv)^2, summed over this core's rows
        y2 = cpool.tile([128, n_rb], f32, tag="y2")
        nc.scalar.activation(y2[:], minv[:], AF.Relu, bias=c_neg_thresh[:], scale=-2.0)
        y2s = cpool.tile([128, n_rb], f32, tag="y2s")
        nc.scalar.activation(y2s[:], y2[:], AF.Square, bias=c_zero[:])
        negsum = cpool.tile([128, 1], f32, tag="negsum")
        nc.vector.reduce_sum(negsum[:], y2s[:], axis=mybir.AxisListType.X)

        outt = cpool.tile([128, 2], f32, tag="outt")
        nc.vector.tensor_copy(outt[:, 0:1], possum[:])
        nc.vector.tensor_copy(outt[:, 1:2], negsum[:])
        nc.sync.dma_start(out[:], outt[:])

    nc.compile()
    return nc


def _split16(v):
    """hi/lo split: v ~= hi + lo with both exactly representable in fp16."""
    hi = v.astype(np.float16)
    lo = (v.astype(np.float32) - hi.astype(np.float32)).astype(np.float16)
    return hi, lo


def _host_prep(F0, F1, positive_pairs, xyz0):
    """Gather + build the augmented GEMM operands (float16)."""
    idx0 = np.asarray(positive_pairs)[:, 0].astype(np.int64)
    idx1 = np.asarray(positive_pairs)[:, 1].astype(np.int64)
    posF0 = np.asarray(F0, dtype=np.float32)[idx0]        # [P, D]
    posF1 = np.asarray(F1, dtype=np.float32)[idx1]        # [P, D]
    sub = np.asarray(xyz0, dtype=np.float32)[idx0]        # [P, 3]
    npairs = posF0.shape[0]

    na = (posF0.astype(np.float64) ** 2).sum(1).astype(np.float32)
    nb = (posF1.astype(np.float64) ** 2).sum(1).astype(np.float32)
    xn = (sub.astype(np.float64) ** 2).sum(1).astype(np.float32)

    # feature GEMM: psum_f[i,j] = d2_ij / 2 = na/2 + nb/2 - f0.f1
    Af = np.concatenate(
        [-posF0, (na / 2)[:, None], np.ones((npairs, 1), np.float32)], axis=1
    )  # [P, 34]
    Bf = np.concatenate(
        [posF1, np.ones((npairs, 1), np.float32), (nb / 2)[:, None]], axis=1
    )  # [P, 34]

    # xyz GEMM: psum_x[i,j] = T2 - d2xyz_ij
    #         = (T2 - xn_i) - xn_j + 2*x_i.x_j, each term hi/lo split in fp16:
    # K layout (13): [2*x_hi(3), 2*x_hi(3), 2*x_lo(3), rowc_hi, rowc_lo, 1, 1]
    #     against   [  y_hi(3),   y_lo(3),   y_hi(3),    1,       1, -xn_hi, -xn_lo]
    x_hi, x_lo = _split16(sub)
    rowc = T2 - xn
    rowc_hi, rowc_lo = _split16(rowc)
    xn_hi, xn_lo = _split16(xn)
    ones = np.ones((npairs, 1), np.float32)
    Ax = np.concatenate(
        [
            2.0 * x_hi.astype(np.float32),
            2.0 * x_hi.astype(np.float32),
            2.0 * x_lo.astype(np.float32),
            rowc_hi.astype(np.float32)[:, None],
            rowc_lo.astype(np.float32)[:, None],
            ones,
            ones,
        ],
        axis=1,
    )  # [P, 13]
    Bx = np.concatenate(
        [
            x_hi.astype(np.float32),
            x_lo.astype(np.float32),
            x_hi.astype(np.float32),
            ones,
            ones,
            -xn_hi.astype(np.float32)[:, None],
            -xn_lo.astype(np.float32)[:, None],
        ],
        axis=1,
    )  # [P, 13]

    pad = np.zeros((npairs, KX - Ax.shape[1]), np.float32)
    Ax = np.concatenate([Ax, pad], axis=1)                # [P, KX]
    Bx = np.concatenate([Bx, pad], axis=1)

    AfT = np.ascontiguousarray(Af.T).astype(np.float16)   # [34, P]
    BfT = np.ascontiguousarray(Bf.T).astype(np.float16)
    AxT = np.ascontiguousarray(Ax.T).astype(np.float16)   # [KX, P]
    BxT = np.ascontiguousarray(Bx.T).astype(np.float16)
    return AfT, BfT, AxT, BxT, posF0, posF1


def _pos_slab(arr, c):
    """[R, D] slab for core c -> [128, (R/128)*D] with row r = rb*128 + p
    mapped to partition p, columns rb*D..rb*D+D."""
    slab = arr[c * R:(c + 1) * R]                          # [R, D]
    return np.ascontiguousarray(
        slab.reshape(R // 128, 128, D).transpose(1, 0, 2).reshape(128, -1)
    ).astype(np.float32)


_LDW_OPT_PATCHED = False


def _enable_ldw_opt():
    """Ask walrus to dedupe/hoist redundant LDWEIGHTS (off by default in
    this harness); correctness is re-checked against the reference on every
    run."""
    global _LDW_OPT_PATCHED
    if _LDW_OPT_PATCHED:
        return
    from concourse import bass_utils as _bu

    _orig = _bu.run_command

    def _patched(cmd, *a, **k):
        if isinstance(cmd, list):
            cmd = [
                "--enable-ldw-opt=true" if c == "--enable-ldw-opt=false" else c
                for c in cmd
            ]
        return _orig(cmd, *a, **k)

    _bu.run_command = _patched
    _LDW_OPT_PATCHED = True


def _ensure_axon_hooks_shim():
    """concourse's trace path imports antenv.axon_hooks, which this image
    lacks; provide a no-op hook module so tracing degrades gracefully."""
    import types

    try:
        import antenv.axon_hooks  # noqa: F401
        return
    except ImportError:
        pass
    try:
        import antenv
    except ImportError:
        return
    mod = types.ModuleType("antenv.axon_hooks")
    _state = {"hook": None}
    mod.set_axon_ntff_profile_hook = lambda h: _state.__setitem__("hook", h)
    mod.get_axon_ntff_profile_hook = lambda: _state["hook"]
    sys.modules["antenv.axon_hooks"] = mod
    antenv.axon_hooks = mod


def kernel(F0, F1, positive_pairs, xyz0):
    from concourse.bass_utils import run_bass_kernel_spmd

    _ensure_axon_hooks_shim()

    AfT, BfT, AxT, BxT, posF0, posF1 = _host_prep(F0, F1, positive_pairs, xyz0)

    nc = _build_program()

    in_maps = []
    for c in range(NCORES):
        in_maps.append(
            {
                "aft": np.ascontiguousarray(AfT[:, c * R:(c + 1) * R]),
                "axt": np.ascontiguousarray(AxT[:, c * R:(c + 1) * R]),
                "bft": BfT,
                "bxt": BxT,
                "pos": np.ascontiguousarray(
                    np.concatenate([_pos_slab(posF0, c), _pos_slab(posF1, c)], axis=1)
                ),
            }
        )

    res = run_bass_kernel_spmd(nc, in_maps, list(range(NCORES)))
    globals()["_LAST_RESULTS"] = res
    total = 0.0
    for r in res.results:
        o = r["out"].astype(np.float64)
        total += o[:, 0].sum() + o[:, 1].sum()
    return np.float32(total / P)
